# revision 1
# baseline (speedup 1.0000x reference)
"""Trainium2 Bass kernel for nn_Decoder_4561255269164 (retrieval_knn).

Math: the reference's top-K(8) KNN collapses to min-reductions:
  - backward: weight w=1/sqrt(d) is nonzero only where d equals the row min
    (over kept candidates), so the scatter-add num/den equals
    E_b^T @ [w*rgb, w] with E_b[i,j] = (d2[i,j] == rowmin_i).
  - forward: only the column argmin rows of d2 matter; sumf/cntf =
    E_f^T @ [rgb, 1] with E_f[i,j] = (d2[i,j] <= colmin_j * (1+1e-6)).
  - exact-match (d==0) rows use a separate weight column gated by rowmin==0.

Sharding: targets (N) split across cores (padded to NT*128 rows each).
Each core computes rowmins for its shard, partial colmins (AllReduce-min),
then indicator matmuls accumulating [12, L] partials (AllReduce-add), and a
redundant O(L) finalize. d2 is computed on the fly by K=5 augmented matmuls
(never stored): s[i,j] = -2*t_i.c_j + a2_i + b2_j, d2 = relu(s); the keep
mask folds into b2 (unkept -> 1e30).
"""

import numpy as np

import concourse.bass as bass
import concourse.bacc as bacc
import concourse.mybir as mybir
import concourse.tile as tile
from concourse import library_config
from concourse.bass_utils import run_bass_kernel_spmd

F32 = mybir.dt.float32
BF16 = mybir.dt.bfloat16
AX = mybir.AxisListType
ALU = mybir.AluOpType
ACTF = mybir.ActivationFunctionType

# geometry (overridable for small-scale simulation tests)
NCORES = 8
L = 16384          # candidates
N = 10000          # targets
NT = 10            # i-tiles of 128 per core (pad 1250 -> 1280)
POINTS_NUM = 8192
BIG = np.float32(1e30)
FWD_EPS = 1.000001  # relative margin for forward colmin match


def _build_nc(reps=1, phases=("AT", "AC", "W", "B", "FIN")):
    nsh = N // NCORES
    npad = NT * 128
    at_w = min(2048, L)           # pass A-T chunk width
    b_w = min(1024, L)            # pass B chunk width
    nq = b_w // 512 if b_w >= 512 else 1

    nc = bacc.Bacc("TRN2", target_bir_lowering=False, debug=False,
                   num_devices=NCORES)

    c5r = nc.declare_dram_parameter("c5r", [5, L], F32, isOutput=False)
    c5m = nc.declare_dram_parameter("c5m", [5, L], F32, isOutput=False)
    ngA_ = max(1, min(2048, L) // 512)
    ngC_ = (npad + 511) // 512
    ngB_ = max(1, min(1024, L) // 512)
    ngmax_ = max(ngA_, ngC_, ngB_)
    ngCB_ = max(ngC_, ngB_)
    t5d = nc.declare_dram_parameter("t5", [5, npad], F32, isOutput=False)
    trgbd = nc.declare_dram_parameter("trgb", [128, NT * 3], F32, isOutput=False)
    rgbpd = nc.declare_dram_parameter("rgbp", [3, L], F32, isOutput=False)
    keepd = nc.declare_dram_parameter("keepf", [1, L], F32, isOutput=False)
    predd = nc.declare_dram_parameter("predf", [1, L], F32, isOutput=False)
    ktgtd = nc.declare_dram_parameter("ktgt", [1, L], F32, isOutput=False)
    eyed = nc.declare_dram_parameter("eye128", [128, 128], F32, isOutput=False)
    chaind = nc.declare_dram_parameter("chain", [1, 2], F32, isOutput=False)
    outd = nc.declare_dram_parameter("out", [1, 2], F32, isOutput=True)

    rg = [list(range(NCORES))]

    with tile.TileContext(nc) as tc:
        nc.gpsimd.load_library(library_config.mlp)
        for _rep in range(reps):
            with (
                tc.tile_pool(name="persist", bufs=1) as pp,
                tc.tile_pool(name="dram", bufs=1, space="DRAM") as dp,
            ):
                ngA = max(1, at_w // 512)
                ngC = (npad + 511) // 512
                ngB = max(1, b_w // 512)
                ngmax = max(ngA, ngC, ngB)
                ngCB = max(ngC, ngB)
                t5s4 = c5a = None
                if {"AT", "AC", "B"} & set(phases):
                    t5s4 = pp.tile([(ngmax - 1) * 32 + 5, npad], F32,
                                   tag="t5s4", name="t5s4")
                    for g in range(ngmax):
                        nc.sync.dma_start(t5s4[32 * g:32 * g + 5, :],
                                          t5d[:, :])
                if {"AC", "B"} & set(phases):
                    c5a = pp.tile([(ngCB - 1) * 32 + 5, L], F32, tag="c5a",
                                  name="c5a")
                    for g in range(ngCB):
                        nc.sync.dma_start(c5a[32 * g:32 * g + 5, :],
                                          c5r[:, :])
                trgb = eye = None
                if "W" in phases:
                    trgb = pp.tile([128, NT * 3], F32, tag="trgb", name="trgb")
                    nc.sync.dma_start(trgb[:], trgbd[:, :])
                if "AC" in phases:
                    eye = pp.tile([128, 128], F32, tag="eye", name="eye")
                    nc.sync.dma_start(eye[:], eyed[:, :])

                m_all = pp.tile([128, NT], F32, tag="m_all")   # raw row mins
                m_relu = pp.tile([128, NT], F32, tag="m_relu")
                m_bf = pp.tile([128, NT], BF16, tag="m_bf")
                m_bf32 = pp.tile([128, NT], F32, tag="m_bf32")
                wb_all = pp.tile([128, NT * 8], BF16, tag="wb_all")
                wf_all = pp.tile([128, NT * 4], BF16, tag="wf_all")
                m2loc = pp.tile([128, L // 128], F32, tag="m2loc")  # [p, jt]

                m2_in = dp.tile([L // 128, 128], F32, tag="m2_in")   # j-linear
                m2_out = dp.tile([1, L], F32, tag="m2_out")
                nd_in = dp.tile([12, L], F32, tag="nd_in")
                nd_out = dp.tile([12, L], F32, tag="nd_out")

                if "AT" in phases:
                    # ---------------- Pass A-T: row mins over kept (masked) ----------
                    with (
                        tc.tile_pool(name="at_cm", bufs=3) as cmp_,
                        tc.tile_pool(name="at_ps", bufs=2, space="PSUM") as psp,
                        tc.tile_pool(name="at_r", bufs=2) as rp,
                    ):
                        nat = L // at_w
                        for t in range(NT):
                            rmin = rp.tile([128, nat], F32, tag="rmin")
                            for jc in range(nat):
                                cm = cmp_.tile([(ngA - 1) * 32 + 5, 512], F32,
                                               tag="cm")
                                for g in range(ngA):
                                    qw = min(512, at_w - g * 512)
                                    nc.sync.dma_start(
                                        cm[32 * g:32 * g + 5, 0:qw],
                                        c5m[:, jc * at_w + g * 512:
                                            jc * at_w + g * 512 + qw])
                                ps = psp.tile([128, at_w], F32, tag="ps")
                                for g in range(ngA):
                                    q0 = g * 512
                                    qw = min(512, at_w - q0)
                                    nc.tensor.matmul(
                                        ps[:, q0:q0 + qw],
                                        lhsT=t5s4[32 * g:32 * g + 5,
                                                  t * 128:(t + 1) * 128],
                                        rhs=cm[32 * g:32 * g + 5, 0:qw],
                                        start=True, stop=True,
                                        tile_position=(32 * g, 0))
                                nc.vector.tensor_reduce(
                                    rmin[:, jc:jc + 1], ps[:], axis=AX.X, op=ALU.min)
                            nc.vector.tensor_reduce(
                                m_all[:, t:t + 1], rmin[:], axis=AX.X, op=ALU.min)

                if "AC" in phases:
                    # ---------------- Pass A-C: local col mins (all targets) ---------
                    with (
                        tc.tile_pool(name="ac_c", bufs=3) as cp2,
                        tc.tile_pool(name="ac_ps", bufs=2, space="PSUM") as psp2,
                    ):
                        nreal = min(nsh, npad)  # pad cols hold 1e30; skip
                        for jt in range(L // 128):
                            ps = psp2.tile([128, npad], F32, tag="ps2")
                            for g in range(ngC):
                                q0 = g * 512
                                qw = min(512, nreal - q0)
                                if qw <= 0:
                                    continue
                                nc.tensor.matmul(
                                    ps[:, q0:q0 + qw],
                                    lhsT=c5a[32 * g:32 * g + 5,
                                             jt * 128:(jt + 1) * 128],
                                    rhs=t5s4[32 * g:32 * g + 5, q0:q0 + qw],
                                    start=True, stop=True,
                                    tile_position=(32 * g, 0))
                            nc.vector.tensor_reduce(
                                m2loc[:, jt:jt + 1], ps[:, 0:nreal],
                                axis=AX.X, op=ALU.min)

                if "AC" in phases:
                    # transpose m2loc -> [jt, p] so DRAM layout is j-linear, then
                    # relu + margin-scale and AllReduce(min).
                    with (
                        tc.tile_pool(name="tr_ps", bufs=1, space="PSUM") as trp,
                        tc.tile_pool(name="tr_sb", bufs=1) as trs,
                    ):
                        pst = trp.tile([128, 128], F32, tag="pst")
                        nc.tensor.transpose(pst[0:L // 128, :], m2loc[:], eye[:])
                        m2t = trs.tile([L // 128, 128], F32, tag="m2t")
                        nc.vector.tensor_scalar(m2t[:], pst[0:L // 128, :], 0.0,
                                                float(FWD_EPS),
                                                op0=ALU.max, op1=ALU.mult)
                        nc.sync.dma_start(m2_in[:, :], m2t[:])
                    if NCORES > 1:
                        nc.gpsimd.collective_compute(
                            "AllReduce", ALU.min, replica_groups=rg,
                            ins=[m2_in.opt()], outs=[m2_out.opt()])
                    else:
                        nc.sync.dma_start(m2_out[0, :],
                                          m2_in[:, :].rearrange("a b -> (a b)"))

                if "W" in phases:
                    # ---------------- weight tiles ----------------------------------
                    with tc.tile_pool(name="wsmall", bufs=1) as ws:
                        nc.vector.tensor_scalar(m_relu[:], m_all[:], 0.0, None,
                                                op0=ALU.max)
                        msafe = ws.tile([128, NT], F32, tag="msafe")
                        nc.vector.tensor_scalar(msafe[:], m_relu[:], 1e-30, None,
                                                op0=ALU.max)
                        sqm = ws.tile([128, NT], F32, tag="sqm")
                        nc.scalar.activation(sqm[:], msafe[:], ACTF.Sqrt)
                        w0 = ws.tile([128, NT], F32, tag="w0")
                        nc.vector.reciprocal(w0[:], sqm[:])
                        vv = ws.tile([128, NT], F32, tag="vv")
                        nc.vector.tensor_scalar(vv[:], m_relu[:], 0.0, None, op0=ALU.is_gt)
                        v2 = ws.tile([128, NT], F32, tag="v2")
                        nc.vector.tensor_scalar(v2[:], m_relu[:], 1e29, None, op0=ALU.is_lt)
                        nc.vector.tensor_tensor(vv[:], vv[:], v2[:], op=ALU.mult)
                        wgt = ws.tile([128, NT], F32, tag="wgt")
                        nc.vector.tensor_tensor(wgt[:], w0[:], vv[:], op=ALU.mult)
                        zz = ws.tile([128, NT], F32, tag="zz")
                        nc.vector.tensor_scalar(zz[:], m_relu[:], 0.0, None,
                                                op0=ALU.is_equal)
                        nc.vector.tensor_copy(m_bf[:], m_relu[:])
                        nc.vector.tensor_copy(m_bf32[:], m_bf[:])

                        wbv = wb_all[:].rearrange("p (t k) -> p t k", k=8)
                        wfv = wf_all[:].rearrange("p (t k) -> p t k", k=4)
                        tv = trgb[:].rearrange("p (t k) -> p t k", k=3)
                        wgv = wgt[:].rearrange("p (t o) -> p t o", o=1)
                        zzv = zz[:].rearrange("p (t o) -> p t o", o=1)
                        for c in range(3):
                            nc.vector.tensor_tensor(
                                wbv[:, :, c:c + 1], wgv, tv[:, :, c:c + 1],
                                op=ALU.mult)
                            nc.vector.tensor_tensor(
                                wbv[:, :, 4 + c:5 + c], zzv, tv[:, :, c:c + 1],
                                op=ALU.mult)
                            nc.vector.tensor_copy(wfv[:, :, c:c + 1],
                                                  tv[:, :, c:c + 1])
                        nc.vector.tensor_copy(wbv[:, :, 3:4], wgv)
                        nc.vector.tensor_copy(wbv[:, :, 7:8], zzv)
                        nc.vector.memset(wfv[:, :, 3:4], 1.0)

                if "B" in phases:
                    # ---------------- Pass B: indicators + scatter matmuls ----------
                    with (
                        tc.tile_pool(name="b_c", bufs=2) as bcp,
                        tc.tile_pool(name="b_m2r", bufs=2) as bm2,
                        tc.tile_pool(name="b_m2b", bufs=2) as bm2b,
                        tc.tile_pool(name="b_d2", bufs=4) as bd2,
                        tc.tile_pool(name="b_e", bufs=4) as bep,
                        tc.tile_pool(name="b_psd", bufs=2, space="PSUM") as bpsd,
                        tc.tile_pool(name="b_acc", bufs=1, space="PSUM") as baccp,
                    ):
                        for jc in range(L // b_w):
                            m2rw = bm2.tile([1, b_w], F32, tag="m2rw")
                            nc.sync.dma_start(m2rw[:],
                                              m2_out[:, jc * b_w:(jc + 1) * b_w])
                            m2rwb = bm2.tile([1, b_w], BF16, tag="m2rwb")
                            nc.vector.tensor_copy(m2rwb[:], m2rw[:])
                            m2b = bm2b.tile([128, b_w], BF16, tag="m2b")
                            nc.gpsimd.partition_broadcast(m2b[:], m2rwb[:])

                            accb = [baccp.tile([8, 512], F32, tag=f"accb{q}",
                                               name=f"accb{q}") for q in range(nq)]
                            accf = [baccp.tile([4, 512], F32, tag=f"accf{q}",
                                               name=f"accf{q}") for q in range(nq)]
                            for t in range(NT):
                                psd = bpsd.tile([128, b_w], F32, tag="psd")
                                for g in range(ngB):
                                    q0 = g * 512
                                    qw = min(512, b_w - q0)
                                    nc.tensor.matmul(
                                        psd[:, q0:q0 + qw],
                                        lhsT=t5s4[32 * g:32 * g + 5,
                                                  t * 128:(t + 1) * 128],
                                        rhs=c5a[32 * g:32 * g + 5,
                                                jc * b_w + q0:jc * b_w + q0 + qw],
                                        start=True, stop=True,
                                        tile_position=(32 * g, 0))
                                d2b = bd2.tile([128, b_w], BF16, tag="d2b")
                                nc.scalar.activation(d2b[:], psd[:], ACTF.Relu)
                                eb = bep.tile([128, b_w], BF16, tag="eb")
                                nc.vector.tensor_scalar(eb[:], d2b[:],
                                                        m_bf32[:, t:t + 1],
                                                        None, op0=ALU.is_equal)
                                ef = bep.tile([128, b_w], BF16, tag="ef")
                                nc.vector.tensor_tensor(ef[:], d2b[:], m2b[:],
                                                        op=ALU.is_le)
                                for q in range(nq):
                                    qw = min(512, b_w - q * 512)
                                    nc.tensor.matmul(
                                        accb[q][:, 0:qw],
                                        lhsT=wb_all[:, t * 8:(t + 1) * 8],
                                        rhs=eb[:, q * 512:q * 512 + qw],
                                        start=(t == 0), stop=(t == NT - 1))
                                    nc.tensor.matmul(
                                        accf[q][:, 0:qw],
                                        lhsT=wf_all[:, t * 4:(t + 1) * 4],
                                        rhs=ef[:, q * 512:q * 512 + qw],
                                        start=(t == 0), stop=(t == NT - 1))
                            for q in range(nq):
                                j0 = jc * b_w + q * 512
                                qw = min(512, b_w - q * 512)
                                ndsb = bep.tile([36, 512], F32, tag="ndsb",
                                                name="ndsb")
                                nc.scalar.copy(ndsb[0:8, 0:qw],
                                               accb[q][:, 0:qw])
                                nc.scalar.copy(ndsb[32:36, 0:qw],
                                               accf[q][:, 0:qw])
                                nc.sync.dma_start(nd_in[0:8, j0:j0 + qw],
                                                  ndsb[0:8, 0:qw])
                                nc.sync.dma_start(nd_in[8:12, j0:j0 + qw],
                                                  ndsb[32:36, 0:qw])
                    if NCORES > 1:
                        nc.gpsimd.collective_compute(
                            "AllReduce", ALU.add, replica_groups=rg,
                            ins=[nd_in.opt()], outs=[nd_out.opt()])
                    else:
                        nc.sync.dma_start(nd_out[:, :], nd_in[:, :])

                if "FIN" in phases:
                    # ---------------- finalize (redundant on every core) ------------
                    lp = L // 128  # plane free width
                    with (
                        tc.tile_pool(name="fin", bufs=1) as fp,
                        tc.tile_pool(name="fin_ps", bufs=1, space="PSUM") as fps,
                    ):
                        def plane_from(dram_row, tg):
                            tl = fp.tile([128, lp], F32, tag=tg, name=tg)
                            nc.sync.dma_start(
                                tl[:], dram_row.rearrange("(p q) -> p q", p=128))
                            return tl

                        nd = [plane_from(nd_out[k, :], f"nd{k}") for k in range(12)]
                        rgbp = [plane_from(rgbpd[k, :], f"rgb{k}") for k in range(3)]
                        keepf = plane_from(keepd[0, :], "keepf")
                        predf = plane_from(predd[0, :], "predf")
                        ktgt = plane_from(ktgtd[0, :], "ktgt")

                        num, den = nd[0:3], nd[3]
                        s0, cnt0 = nd[4:7], nd[7]
                        sf, cntf = nd[8:11], nd[11]

                        _cnt = [0]

                        def newt():
                            _cnt[0] += 1
                            return fp.tile([128, lp], F32, tag=f"fin{_cnt[0]}",
                                           name=f"fin{_cnt[0]}")

                        dsafe = newt()
                        nc.vector.tensor_scalar(dsafe[:], den[:], 0.0, None,
                                                op0=ALU.is_equal)
                        nc.vector.tensor_tensor(dsafe[:], dsafe[:], den[:], op=ALU.add)
                        rden = newt()
                        nc.vector.reciprocal(rden[:], dsafe[:])
                        c0safe = newt()
                        nc.vector.tensor_scalar(c0safe[:], cnt0[:], 0.0, None,
                                                op0=ALU.is_equal)
                        nc.vector.tensor_tensor(c0safe[:], c0safe[:], cnt0[:],
                                                op=ALU.add)
                        rcnt0 = newt()
                        nc.vector.reciprocal(rcnt0[:], c0safe[:])
                        rcntf = newt()
                        nc.vector.reciprocal(rcntf[:], cntf[:])

                        mden = fp.tile([128, lp], mybir.dt.int32, tag="mden",
                                       name="mden")
                        nc.vector.tensor_scalar(mden[:], den[:], 0.0, None,
                                                op0=ALU.not_equal)
                        mz = fp.tile([128, lp], mybir.dt.int32, tag="mz", name="mz")
                        nc.vector.tensor_scalar(mz[:], cnt0[:], 0.0, None,
                                                op0=ALU.is_gt)

                        acc = newt()
                        nc.vector.memset(acc[:], 0.0)
                        for c in range(3):
                            rec = newt()
                            nc.vector.tensor_tensor(rec[:], sf[c][:], rcntf[:],
                                                    op=ALU.mult)
                            tmp = newt()
                            nc.vector.tensor_tensor(tmp[:], num[c][:], rden[:],
                                                    op=ALU.mult)
                            nc.vector.copy_predicated(rec[:], mden[:], tmp[:])
                            nc.vector.tensor_tensor(tmp[:], s0[c][:], rcnt0[:],
                                                    op=ALU.mult)
                            nc.vector.copy_predicated(rec[:], mz[:], tmp[:])
                            diff = newt()
                            nc.vector.tensor_tensor(diff[:], rgbp[c][:], rec[:],
                                                    op=ALU.subtract)
                            ad = newt()
                            nc.scalar.activation(ad[:], diff[:], ACTF.Abs)
                            nc.vector.tensor_tensor(acc[:], acc[:], ad[:], op=ALU.add)
                        nc.vector.tensor_tensor(acc[:], acc[:], keepf[:], op=ALU.mult)

                        # BCE: relu(p) - p*t + softplus(-|p|)
                        bce = newt()
                        nc.scalar.activation(bce[:], predf[:], ACTF.Relu)
                        pt = newt()
                        nc.vector.tensor_tensor(pt[:], predf[:], ktgt[:], op=ALU.mult)
                        nc.vector.tensor_tensor(bce[:], bce[:], pt[:], op=ALU.subtract)
                        ap_ = newt()
                        nc.scalar.activation(ap_[:], predf[:], ACTF.Abs)
                        en = newt()
                        nc.scalar.activation(en[:], ap_[:], ACTF.Exp, scale=-1.0)
                        sp = newt()
                        nc.scalar.activation(sp[:], en[:], ACTF.Ln, bias=1.0)
                        nc.vector.tensor_tensor(bce[:], bce[:], sp[:], op=ALU.add)

                        rows2 = fp.tile([128, 2], F32, tag="rows2")
                        nc.vector.tensor_reduce(rows2[:, 0:1], bce[:], axis=AX.X,
                                                op=ALU.add)
                        nc.vector.tensor_reduce(rows2[:, 1:2], acc[:], axis=AX.X,
                                                op=ALU.add)
                        onescol = fp.tile([128, 1], F32, tag="onescol")
                        nc.vector.memset(onescol[:], 1.0)
                        pstot = fps.tile([1, 2], F32, tag="pstot")
                        nc.tensor.matmul(pstot[:], lhsT=onescol[:], rhs=rows2[:],
                                         start=True, stop=True)
                        chsb = fp.tile([1, 2], F32, tag="chsb")
                        nc.sync.dma_start(chsb[:], chaind[:, :])
                        nc.vector.tensor_scalar(chsb[:], chsb[:], 0.0, None,
                                                op0=ALU.mult)
                        outsb = fp.tile([1, 2], F32, tag="outsb")
                        nc.scalar.copy(outsb[:], pstot[:])
                        nc.vector.tensor_tensor(outsb[:], outsb[:], chsb[:],
                                                op=ALU.add)
                        nc.sync.dma_start(outd[:, :], outsb[:])

    nc.compile()
    return nc


def _host_prep(pred_F, cand_xyz, cand_rgb, tgt_xyz, tgt_rgb, keep_target,
               points_num):
    nsh = N // NCORES
    npad = NT * 128
    pred = np.ascontiguousarray(np.asarray(pred_F, np.float32))
    cxyz = np.ascontiguousarray(np.asarray(cand_xyz, np.float32))
    crgb = np.ascontiguousarray(np.asarray(cand_rgb, np.float32))
    txyz = np.ascontiguousarray(np.asarray(tgt_xyz, np.float32))
    trgb_np = np.ascontiguousarray(np.asarray(tgt_rgb, np.float32))
    ktgt = np.asarray(keep_target).astype(np.float32)

    # keep mask (exact reference semantics, f32)
    p8 = pred.reshape(-1, 8)
    rows = np.arange(p8.shape[0])
    ilm = np.zeros(p8.shape, dtype=bool)
    ilm[rows, np.argmax(p8, axis=1)] = True
    ilm = ilm.reshape(-1)
    k = L - int(points_num)
    vals = np.where(ilm, np.inf, pred)
    thr = np.sort(vals)[k - 1]
    keep = (pred > thr) | ilm

    b2 = np.sum(cxyz * cxyz, axis=1, dtype=np.float32).astype(np.float32)
    b2m = np.where(keep, b2, BIG).astype(np.float32)
    ones = np.ones(L, np.float32)
    c5r = np.ascontiguousarray(
        np.stack([cxyz[:, 0], cxyz[:, 1], cxyz[:, 2], ones, b2]))
    c5m = np.ascontiguousarray(
        np.stack([cxyz[:, 0], cxyz[:, 1], cxyz[:, 2], ones, b2m]))

    a2 = np.sum(txyz * txyz, axis=1, dtype=np.float32).astype(np.float32)

    ngA = max(1, min(2048, L) // 512)
    ngC = (npad + 511) // 512
    ngB = max(1, min(1024, L) // 512)
    ngmax = max(ngA, ngC, ngB)
    ngCB = max(ngC, ngB)

    def repl(a, ng):
        out = np.zeros(((ng - 1) * 32 + a.shape[0], a.shape[1]), np.float32)
        for g in range(ng):
            out[32 * g:32 * g + a.shape[0]] = a
        return np.ascontiguousarray(out)

    t5_cores, trgb_cores = [], []
    for c in range(NCORES):
        sl = slice(c * nsh, (c + 1) * nsh)
        t5 = np.zeros((5, npad), np.float32)
        t5[3, :] = BIG     # pad rows: s = 1e30 everywhere
        t5[4, :] = 1.0
        t5[0, :nsh] = -2.0 * txyz[sl, 0]
        t5[1, :nsh] = -2.0 * txyz[sl, 1]
        t5[2, :nsh] = -2.0 * txyz[sl, 2]
        t5[3, :nsh] = a2[sl]
        tr = np.zeros((npad, 3), np.float32)
        tr[:nsh] = trgb_np[sl]
        # [p, t*3+c] layout: target i_local = t*128 + p
        trc = tr.reshape(NT, 128, 3).transpose(1, 0, 2).reshape(128, NT * 3)
        t5_cores.append(np.ascontiguousarray(t5))
        trgb_cores.append(np.ascontiguousarray(trc))

    rgbp = np.ascontiguousarray((crgb * np.float32(255.0)).T.astype(np.float32))
    keepf = keep.astype(np.float32).reshape(1, L)
    eye = np.eye(128, dtype=np.float32)

    common = dict(c5r=c5r, c5m=c5m, rgbp=rgbp,
                  keepf=keepf, predf=pred.reshape(1, L),
                  ktgt=ktgt.reshape(1, L), eye128=eye,
                  chain=np.zeros((1, 2), np.float32))
    in_maps = [dict(common, t5=t5_cores[c], trgb=trgb_cores[c])
               for c in range(NCORES)]
    return in_maps


_CACHE = {}


def kernel(pred_F, cand_xyz, cand_rgb, tgt_xyz, tgt_rgb, keep_target,
           points_num=8192, **_ignored):
    in_maps = _host_prep(pred_F, cand_xyz, cand_rgb, tgt_xyz, tgt_rgb,
                         keep_target, points_num)
    if "nc" not in _CACHE:
        _CACHE["nc"] = _build_nc()
    res = run_bass_kernel_spmd(_CACHE["nc"], in_maps,
                               core_ids=list(range(NCORES)))
    return np.asarray(res.results[0]["out"], np.float32).reshape(2)


if __name__ == "__main__":
    import reference as R
    inputs = R.setup_inputs()
    inputs = {kk: np.asarray(vv) if not np.isscalar(vv) else vv
              for kk, vv in inputs.items()}
    out = kernel(**inputs)
    print("kernel out:", out)



# revision 4
# speedup vs baseline: 4.3058x; 4.3058x over previous
"""Trainium2 Bass kernel for nn_Decoder_4561255269164 (retrieval_knn).

Math: the reference's top-K(8) KNN collapses to min-reductions:
  - backward: weight w=1/sqrt(d) is nonzero only where d equals the row min
    (over kept candidates), so the scatter-add num/den equals
    E_b^T @ [w*rgb, w] with E_b[i,j] = (d2[i,j] == rowmin_i).
  - forward: only the column argmin rows of d2 matter; sf/cntf =
    E_f^T @ [rgb, 1] with E_f[i,j] = (d2[i,j] <= colmin_j).
  - exact-match (d==0) rows use a separate weight column gated by rowmin==0.

Key optimizations over the fp32 3-pass version:
  - Only KEPT candidates (exactly points_num = 8192 of 16384) participate in
    the whole recolor loss, so candidates are compacted host-side to KC=8192
    columns. Halves every pass.
  - d2 is computed in bf16 matmuls (1 cycle/col vs fp32's 4) with hi/lo split
    coordinates (16 contract rows) for ~1e-2 absolute accuracy; both passes
    compute bitwise-identical values so equality compares need no epsilon.
  - Pass A computes row mins AND column mins in one sweep: the Act engine
    copies PSUM to negated bf16, DVE max-folds rows and columns, gpsimd
    partition_all_reduce(max) collapses partitions for the column mins.
  - Column-min AllReduce fires in two halves early (jcg-outer loop) so it
    hides under pass A/B compute.
  - Scatter matmuls run concurrently via col-group tile_position, distance
    matmuls via row-group tile_position packing.
  - nd reduction uses ReduceScatter + per-core sharded finalize; the host
    sums the 8 per-core partial [coord_loss, rgb_loss] outputs.
"""

import numpy as np

import concourse.bass as bass
import concourse.bass_isa as bass_isa
import concourse.bacc as bacc
import concourse.mybir as mybir
import concourse.tile as tile
from concourse import library_config
from concourse.bass_utils import run_bass_kernel_spmd

F32 = mybir.dt.float32
BF16 = mybir.dt.bfloat16
AX = mybir.AxisListType
ALU = mybir.AluOpType
ACTF = mybir.ActivationFunctionType
NPBF = mybir.dt.np(BF16)

# geometry
NCORES = 8
L = 16384          # candidate voxels (full)
N = 10000          # targets
NSH = N // NCORES  # targets per core (1250)
NT = 10            # i-tiles of 128 per core (pad 1250 -> 1280)
NPAD = NT * 128
KC = 8192          # compacted (kept) candidate columns = points_num
SHW = KC // NCORES  # finalize shard width per core (1024)
LSH = L // NCORES   # BCE shard width per core (2048)
BIG = np.float32(1e30)

CWA = 2048         # pass A chunk width (4 x 512 row-group-packed matmuls)
GA = 4
NJA = KC // CWA    # 4
CWB = 1024         # pass B chunk width (2 x 512)
GB = 2
NJB = KC // CWB    # 8
NMETA = 4          # pass B meta-passes (2 j-groups each, PSUM-limited)


def _build_nc(reps=1, phases=("A", "W", "B", "FIN")):
    nc = bacc.Bacc("TRN2", target_bir_lowering=False, debug=False,
                   num_devices=NCORES)

    c16d = nc.declare_dram_parameter("c16", [16, KC], BF16, isOutput=False)
    t16d = nc.declare_dram_parameter("t16", [16, NPAD], BF16, isOutput=False)
    trgbd = nc.declare_dram_parameter("trgb", [128, NT * 3], F32,
                                      isOutput=False)
    rgbshd = nc.declare_dram_parameter("rgbsh", [3, SHW], F32, isOutput=False)
    keepshd = nc.declare_dram_parameter("keepsh", [1, SHW], F32,
                                        isOutput=False)
    predshd = nc.declare_dram_parameter("predsh", [1, LSH], F32,
                                        isOutput=False)
    ktgtshd = nc.declare_dram_parameter("ktgtsh", [1, LSH], F32,
                                        isOutput=False)
    chaind = nc.declare_dram_parameter("chain", [1, 2], F32, isOutput=False)
    outd = nc.declare_dram_parameter("out", [1, 2], F32, isOutput=True)

    rg = [list(range(NCORES))]

    with tile.TileContext(nc) as tc:
        nc.gpsimd.load_library(library_config.mlp)
        for _rep in range(reps):
            with (
                tc.tile_pool(name="persist", bufs=1) as pp,
                tc.tile_pool(name="dram", bufs=1, space="DRAM") as dp,
            ):
                # ---- persistent SBUF state ----
                Cr = pp.tile([(GA - 1) * 32 + 16, KC], BF16, tag="Cr",
                             name="Cr")
                T5r = pp.tile([(GA - 1) * 32 + 16, NPAD], BF16, tag="T5r",
                              name="T5r")
                for g in range(GA):
                    nc.sync.dma_start(Cr[32 * g:32 * g + 16, :], c16d[:, :])
                    nc.sync.dma_start(T5r[32 * g:32 * g + 16, :], t16d[:, :])
                trgbs = pp.tile([128, NT * 3], F32, tag="trgbs", name="trgbs")
                nc.sync.dma_start(trgbs[:], trgbd[:, :])

                colneg = pp.tile([128, KC], BF16, tag="colneg", name="colneg")
                rowneg = pp.tile([128, NT * 1024], BF16, tag="rowneg",
                                 name="rowneg")
                m2ball = pp.tile([128, KC], BF16, tag="m2ball", name="m2ball")
                m_bf = pp.tile([128, NT], F32, tag="m_bf")
                wb_all = pp.tile([128, NT * 8], BF16, tag="wb_all")
                wf_all = pp.tile([128, NT * 4], BF16, tag="wf_all")

                m2i = [dp.tile([1, 2 * CWA], BF16, tag=f"m2i{h}",
                               name=f"m2i{h}") for h in range(2)]
                m2o = [dp.tile([1, 2 * CWA], BF16, tag=f"m2o{h}",
                               name=f"m2o{h}") for h in range(2)]
                ndi = dp.tile([NJB * 12, SHW], F32, tag="ndi", name="ndi")
                ndo = dp.tile([12, SHW], F32, tag="ndo", name="ndo")

                if "A" in phases:
                    # ------- pass A: d2 sweep -> row mins + col mins -------
                    nc.vector.memset(rowneg[:], -float(BIG))
                    with (
                        tc.tile_pool(name="a_ps", bufs=2, space="PSUM") as psa,
                        tc.tile_pool(name="a_nd", bufs=3) as nda,
                        tc.tile_pool(name="a_cm", bufs=2) as cma,
                        tc.tile_pool(name="a_row", bufs=1) as rpa,
                    ):
                        for jcg in range(NJA):
                            j0 = jcg * CWA
                            for t in range(NT):
                                ps = psa.tile([128, CWA], F32, tag="psA")
                                for g in range(GA):
                                    nc.tensor.matmul(
                                        ps[:, 512 * g:512 * (g + 1)],
                                        lhsT=T5r[32 * g:32 * g + 16,
                                                 128 * t:128 * (t + 1)],
                                        rhs=Cr[32 * g:32 * g + 16,
                                               j0 + 512 * g:j0 + 512 * (g + 1)],
                                        start=True, stop=True,
                                        tile_position=(32 * g, 0))
                                nd2 = nda.tile([128, CWA], BF16, tag="nd2")
                                nc.scalar.activation(nd2[:], ps[:], ACTF.Copy,
                                                     scale=-1.0)
                                rsl = rowneg[:, t * 1024:(t + 1) * 1024]
                                nc.vector.tensor_tensor(
                                    rsl, rsl, nd2[:, 0:1024], op=ALU.max)
                                nc.vector.tensor_tensor(
                                    rsl, rsl, nd2[:, 1024:2048], op=ALU.max)
                                csl = colneg[:, j0:j0 + CWA]
                                if t == 0:
                                    nc.vector.tensor_copy(csl, nd2[:])
                                else:
                                    nc.vector.tensor_tensor(csl, csl, nd2[:],
                                                            op=ALU.max)
                            # col-min (negated -> max) across partitions
                            cm = cma.tile([128, CWA], BF16, tag="cmA")
                            nc.gpsimd.partition_all_reduce(
                                cm[:], colneg[:, j0:j0 + CWA], 128,
                                bass_isa.ReduceOp.max)
                            half, piece = jcg // 2, jcg % 2
                            nc.sync.dma_start(
                                m2i[half][0:1, piece * CWA:(piece + 1) * CWA],
                                cm[0:1, :])
                            if piece == 1:
                                if NCORES > 1:
                                    nc.gpsimd.collective_compute(
                                        "AllReduce", ALU.max,
                                        replica_groups=rg,
                                        ins=[m2i[half].opt()],
                                        outs=[m2o[half].opt()])
                                else:
                                    nc.sync.dma_start(m2o[half][:, :],
                                                      m2i[half][:, :])
                                row = rpa.tile([1, 2 * CWA], BF16,
                                               tag=f"m2r{half}",
                                               name=f"m2r{half}")
                                nc.sync.dma_start(row[:], m2o[half][:, :])
                                rowp = rpa.tile([1, 2 * CWA], BF16,
                                                tag=f"m2p{half}",
                                                name=f"m2p{half}")
                                nc.vector.tensor_scalar(rowp[:], row[:], -1.0,
                                                        None, op0=ALU.mult)
                                nc.gpsimd.partition_broadcast(
                                    m2ball[:, half * 2 * CWA:
                                           (half + 1) * 2 * CWA], rowp[:])

                if "W" in phases:
                    # ------- row-min finalize + weight tiles -------
                    with tc.tile_pool(name="wsmall", bufs=1) as ws:
                        rmaxn = ws.tile([128, NT], F32, tag="rmaxn")
                        for t in range(NT):
                            nc.vector.tensor_reduce(
                                rmaxn[:, t:t + 1],
                                rowneg[:, t * 1024:(t + 1) * 1024],
                                axis=AX.X, op=ALU.max)
                        m_all = ws.tile([128, NT], F32, tag="m_all")
                        nc.vector.tensor_scalar(m_all[:], rmaxn[:], -1.0,
                                                None, op0=ALU.mult)
                        nc.vector.tensor_copy(m_bf[:], m_all[:])
                        m_relu = ws.tile([128, NT], F32, tag="m_relu")
                        nc.vector.tensor_scalar(m_relu[:], m_all[:], 0.0,
                                                None, op0=ALU.max)
                        msafe = ws.tile([128, NT], F32, tag="msafe")
                        nc.vector.tensor_scalar(msafe[:], m_relu[:], 1e-30,
                                                None, op0=ALU.max)
                        sqm = ws.tile([128, NT], F32, tag="sqm")
                        nc.scalar.activation(sqm[:], msafe[:], ACTF.Sqrt)
                        w0 = ws.tile([128, NT], F32, tag="w0")
                        nc.vector.reciprocal(w0[:], sqm[:])
                        vv = ws.tile([128, NT], F32, tag="vv")
                        nc.vector.tensor_scalar(vv[:], m_relu[:], 0.0, None,
                                                op0=ALU.is_gt)
                        v2 = ws.tile([128, NT], F32, tag="v2")
                        nc.vector.tensor_scalar(v2[:], m_relu[:], 1e29, None,
                                                op0=ALU.is_lt)
                        nc.vector.tensor_tensor(vv[:], vv[:], v2[:],
                                                op=ALU.mult)
                        wgt = ws.tile([128, NT], F32, tag="wgt")
                        nc.vector.tensor_tensor(wgt[:], w0[:], vv[:],
                                                op=ALU.mult)
                        zz = ws.tile([128, NT], F32, tag="zz")
                        nc.vector.tensor_scalar(zz[:], m_relu[:], 0.0, None,
                                                op0=ALU.is_equal)

                        wbv = wb_all[:].rearrange("p (t k) -> p t k", k=8)
                        wfv = wf_all[:].rearrange("p (t k) -> p t k", k=4)
                        tv = trgbs[:].rearrange("p (t k) -> p t k", k=3)
                        wgv = wgt[:].rearrange("p (t o) -> p t o", o=1)
                        zzv = zz[:].rearrange("p (t o) -> p t o", o=1)
                        for c in range(3):
                            nc.vector.tensor_tensor(
                                wbv[:, :, c:c + 1], wgv, tv[:, :, c:c + 1],
                                op=ALU.mult)
                            nc.vector.tensor_tensor(
                                wbv[:, :, 4 + c:5 + c], zzv, tv[:, :, c:c + 1],
                                op=ALU.mult)
                            nc.vector.tensor_copy(wfv[:, :, c:c + 1],
                                                  tv[:, :, c:c + 1])
                        nc.vector.tensor_copy(wbv[:, :, 3:4], wgv)
                        nc.vector.tensor_copy(wbv[:, :, 7:8], zzv)
                        nc.vector.memset(wfv[:, :, 3:4], 1.0)

                if "B" in phases:
                    # ------- pass B: indicators + scatter matmuls -------
                    with (
                        tc.tile_pool(name="b_ps", bufs=2, space="PSUM") as psb,
                        tc.tile_pool(name="b_acc", bufs=1,
                                     space="PSUM") as accp,
                        tc.tile_pool(name="b_d2", bufs=3) as dbp,
                        tc.tile_pool(name="b_e", bufs=4) as ebp,
                    ):
                        for meta in range(NMETA):
                            accs = [accp.tile([36, CWB], F32, tag=f"acc{q}",
                                              name=f"acc{q}")
                                    for q in range(2)]
                            for t in range(NT):
                                for q in range(2):
                                    jc = meta * 2 + q
                                    j0 = jc * CWB
                                    ps = psb.tile([128, CWB], F32, tag="psB")
                                    for g in range(GB):
                                        nc.tensor.matmul(
                                            ps[:, 512 * g:512 * (g + 1)],
                                            lhsT=T5r[32 * g:32 * g + 16,
                                                     128 * t:128 * (t + 1)],
                                            rhs=Cr[32 * g:32 * g + 16,
                                                   j0 + 512 * g:
                                                   j0 + 512 * (g + 1)],
                                            start=True, stop=True,
                                            tile_position=(32 * g, 0))
                                    d2b = dbp.tile([128, CWB], BF16,
                                                   tag="d2b")
                                    nc.scalar.activation(d2b[:], ps[:],
                                                         ACTF.Copy)
                                    eb = ebp.tile([128, CWB], BF16, tag="eb")
                                    nc.vector.tensor_scalar(
                                        eb[:], d2b[:], m_bf[:, t:t + 1],
                                        None, op0=ALU.is_equal)
                                    ef = ebp.tile([128, CWB], BF16, tag="ef")
                                    nc.vector.tensor_tensor(
                                        ef[:], d2b[:],
                                        m2ball[:, j0:j0 + CWB], op=ALU.is_le)
                                    for h in range(2):
                                        hs = slice(512 * h, 512 * (h + 1))
                                        nc.tensor.matmul(
                                            accs[q][0:8, hs],
                                            lhsT=wb_all[:, 8 * t:8 * (t + 1)],
                                            rhs=eb[:, hs],
                                            start=(t == 0),
                                            stop=(t == NT - 1),
                                            tile_position=(0, 0))
                                        nc.tensor.matmul(
                                            accs[q][32:36, hs],
                                            lhsT=wf_all[:, 4 * t:4 * (t + 1)],
                                            rhs=ef[:, hs],
                                            start=(t == 0),
                                            stop=(t == NT - 1),
                                            tile_position=(0, 32))
                            for q in range(2):
                                jc = meta * 2 + q
                                ndsb = ebp.tile([36, CWB], F32, tag="ndsb",
                                                name="ndsb")
                                nc.scalar.copy(ndsb[:], accs[q][:])
                                nc.sync.dma_start(
                                    ndi[jc * 12:jc * 12 + 8, :],
                                    ndsb[0:8, :])
                                nc.sync.dma_start(
                                    ndi[jc * 12 + 8:jc * 12 + 12, :],
                                    ndsb[32:36, :])
                    if NCORES > 1:
                        nc.gpsimd.collective_compute(
                            "ReduceScatter", ALU.add, replica_groups=rg,
                            ins=[ndi.opt()], outs=[ndo.opt()])
                    else:
                        nc.sync.dma_start(ndo[:, :], ndi[0:12, :])

                if "FIN" in phases:
                    # ------- per-core shard finalize -------
                    lp = SHW // 128   # 8
                    lp2 = LSH // 128  # 16
                    with (
                        tc.tile_pool(name="fin", bufs=1) as fp,
                        tc.tile_pool(name="fin_ps", bufs=1,
                                     space="PSUM") as fps,
                    ):
                        def plane_from(dram_row, tg, w):
                            tl = fp.tile([128, w], F32, tag=tg, name=tg)
                            nc.sync.dma_start(
                                tl[:], dram_row.rearrange("(p q) -> p q",
                                                          p=128))
                            return tl

                        nd = [plane_from(ndo[k, :], f"nd{k}", lp)
                              for k in range(12)]
                        rgbp = [plane_from(rgbshd[k, :], f"rgb{k}", lp)
                                for k in range(3)]
                        keepf = plane_from(keepshd[0, :], "keepf", lp)
                        predf = plane_from(predshd[0, :], "predf", lp2)
                        ktgt = plane_from(ktgtshd[0, :], "ktgt", lp2)

                        num, den = nd[0:3], nd[3]
                        s0, cnt0 = nd[4:7], nd[7]
                        sf, cntf = nd[8:11], nd[11]

                        _cnt = [0]

                        def newt(w=lp):
                            _cnt[0] += 1
                            return fp.tile([128, w], F32,
                                           tag=f"fin{_cnt[0]}",
                                           name=f"fin{_cnt[0]}")

                        dsafe = newt()
                        nc.vector.tensor_scalar(dsafe[:], den[:], 0.0, None,
                                                op0=ALU.is_equal)
                        nc.vector.tensor_tensor(dsafe[:], dsafe[:], den[:],
                                                op=ALU.add)
                        rden = newt()
                        nc.vector.reciprocal(rden[:], dsafe[:])
                        c0safe = newt()
                        nc.vector.tensor_scalar(c0safe[:], cnt0[:], 0.0, None,
                                                op0=ALU.is_equal)
                        nc.vector.tensor_tensor(c0safe[:], c0safe[:],
                                                cnt0[:], op=ALU.add)
                        rcnt0 = newt()
                        nc.vector.reciprocal(rcnt0[:], c0safe[:])
                        cfsafe = newt()
                        nc.vector.tensor_scalar(cfsafe[:], cntf[:], 0.0, None,
                                                op0=ALU.is_equal)
                        nc.vector.tensor_tensor(cfsafe[:], cfsafe[:],
                                                cntf[:], op=ALU.add)
                        rcntf = newt()
                        nc.vector.reciprocal(rcntf[:], cfsafe[:])

                        mden = fp.tile([128, lp], mybir.dt.int32, tag="mden",
                                       name="mden")
                        nc.vector.tensor_scalar(mden[:], den[:], 0.0, None,
                                                op0=ALU.not_equal)
                        mz = fp.tile([128, lp], mybir.dt.int32, tag="mz",
                                     name="mz")
                        nc.vector.tensor_scalar(mz[:], cnt0[:], 0.0, None,
                                                op0=ALU.is_gt)

                        acc = newt()
                        nc.vector.memset(acc[:], 0.0)
                        for c in range(3):
                            rec = newt()
                            nc.vector.tensor_tensor(rec[:], sf[c][:],
                                                    rcntf[:], op=ALU.mult)
                            tmp = newt()
                            nc.vector.tensor_tensor(tmp[:], num[c][:],
                                                    rden[:], op=ALU.mult)
                            nc.vector.copy_predicated(rec[:], mden[:], tmp[:])
                            nc.vector.tensor_tensor(tmp[:], s0[c][:],
                                                    rcnt0[:], op=ALU.mult)
                            nc.vector.copy_predicated(rec[:], mz[:], tmp[:])
                            diff = newt()
                            nc.vector.tensor_tensor(diff[:], rgbp[c][:],
                                                    rec[:], op=ALU.subtract)
                            ad = newt()
                            nc.scalar.activation(ad[:], diff[:], ACTF.Abs)
                            nc.vector.tensor_tensor(acc[:], acc[:], ad[:],
                                                    op=ALU.add)
                        nc.vector.tensor_tensor(acc[:], acc[:], keepf[:],
                                                op=ALU.mult)

                        # BCE: relu(p) - p*t + softplus(-|p|)
                        bce = newt(lp2)
                        nc.scalar.activation(bce[:], predf[:], ACTF.Relu)
                        pt = newt(lp2)
                        nc.vector.tensor_tensor(pt[:], predf[:], ktgt[:],
                                                op=ALU.mult)
                        nc.vector.tensor_tensor(bce[:], bce[:], pt[:],
                                                op=ALU.subtract)
                        ap_ = newt(lp2)
                        nc.scalar.activation(ap_[:], predf[:], ACTF.Abs)
                        en = newt(lp2)
                        nc.scalar.activation(en[:], ap_[:], ACTF.Exp,
                                             scale=-1.0)
                        sp = newt(lp2)
                        nc.scalar.activation(sp[:], en[:], ACTF.Ln, bias=1.0)
                        nc.vector.tensor_tensor(bce[:], bce[:], sp[:],
                                                op=ALU.add)

                        rows2 = fp.tile([128, 2], F32, tag="rows2")
                        nc.vector.tensor_reduce(rows2[:, 0:1], bce[:],
                                                axis=AX.X, op=ALU.add)
                        nc.vector.tensor_reduce(rows2[:, 1:2], acc[:],
                                                axis=AX.X, op=ALU.add)
                        onescol = fp.tile([128, 1], F32, tag="onescol")
                        nc.vector.memset(onescol[:], 1.0)
                        pstot = fps.tile([1, 2], F32, tag="pstot")
                        nc.tensor.matmul(pstot[:], lhsT=onescol[:],
                                         rhs=rows2[:], start=True, stop=True)
                        chsb = fp.tile([1, 2], F32, tag="chsb")
                        nc.sync.dma_start(chsb[:], chaind[:, :])
                        nc.vector.tensor_scalar(chsb[:], chsb[:], 0.0, None,
                                                op0=ALU.mult)
                        outsb = fp.tile([1, 2], F32, tag="outsb")
                        nc.scalar.copy(outsb[:], pstot[:])
                        nc.vector.tensor_tensor(outsb[:], outsb[:], chsb[:],
                                                op=ALU.add)
                        nc.sync.dma_start(outd[:, :], outsb[:])

    nc.compile()
    return nc


def _bfsplit(x):
    """f32 array -> (hi, lo) bf16 arrays with hi + lo ~= x."""
    x = np.asarray(x, np.float32)
    hi = x.astype(NPBF)
    lo = (x - hi.astype(np.float32)).astype(NPBF)
    return hi, lo


def _host_prep(pred_F, cand_xyz, cand_rgb, tgt_xyz, tgt_rgb, keep_target,
               points_num):
    pred = np.ascontiguousarray(np.asarray(pred_F, np.float32))
    cxyz = np.ascontiguousarray(np.asarray(cand_xyz, np.float32))
    crgb = np.ascontiguousarray(np.asarray(cand_rgb, np.float32))
    txyz = np.ascontiguousarray(np.asarray(tgt_xyz, np.float32))
    trgb_np = np.ascontiguousarray(np.asarray(tgt_rgb, np.float32))
    ktgt = np.asarray(keep_target).astype(np.float32)

    # keep mask (exact reference semantics)
    p8 = pred.reshape(-1, 8)
    rows = np.arange(p8.shape[0])
    ilm = np.zeros(p8.shape, dtype=bool)
    ilm[rows, np.argmax(p8, axis=1)] = True
    ilm = ilm.reshape(-1)
    k = L - int(points_num)
    vals = np.where(ilm, np.inf, pred)
    thr = np.sort(vals)[k - 1]
    keep = (pred > thr) | ilm

    kidx = np.nonzero(keep)[0]
    nk = len(kidx)
    if nk > KC:  # only possible with pred ties; drop extras (tiny loss shift)
        kidx = kidx[:KC]
        nk = KC

    cx = cxyz[kidx]                       # [nk, 3]
    ch, cl = _bfsplit(cx)
    b2 = np.sum(cx * cx, axis=1, dtype=np.float32).astype(np.float32)
    bh, bl = _bfsplit(b2)
    C = np.zeros((16, KC), NPBF)
    C[0:3, :nk] = ch.T
    C[3:6, :nk] = ch.T
    C[6:9, :nk] = cl.T
    C[9:12, :nk] = cl.T
    C[12, :nk] = np.float32(1.0)
    C[13, :nk] = np.float32(1.0)
    C[14, :nk] = bh
    C[15, :nk] = bl
    C[14, nk:] = BIG  # pad columns: s = 1e30 (row 14 pairs with T=ones)

    rgbk = np.zeros((3, KC), np.float32)
    rgbk[:, :nk] = (crgb[kidx] * np.float32(255.0)).T
    keepk = np.zeros((1, KC), np.float32)
    keepk[0, :nk] = 1.0

    common = dict(c16=np.ascontiguousarray(C),
                  chain=np.zeros((1, 2), np.float32))

    in_maps = []
    for c in range(NCORES):
        sl = slice(c * NSH, (c + 1) * NSH)
        tc_ = txyz[sl]
        th, tl = _bfsplit(tc_)
        a2 = np.sum(tc_ * tc_, axis=1, dtype=np.float32).astype(np.float32)
        ah, al = _bfsplit(a2)
        T = np.zeros((16, NPAD), NPBF)
        T[0:3, :NSH] = (-2.0 * th.astype(np.float32)).astype(NPBF).T
        T[3:6, :NSH] = (-2.0 * tl.astype(np.float32)).astype(NPBF).T
        T[6:9, :NSH] = T[0:3, :NSH]
        T[9:12, :NSH] = T[3:6, :NSH]
        T[12, :NSH] = ah
        T[13, :NSH] = al
        T[14, :NSH] = np.float32(1.0)
        T[15, :NSH] = np.float32(1.0)
        T[12, NSH:] = BIG  # pad target rows: s = 1e30 everywhere

        tr = np.zeros((NPAD, 3), np.float32)
        tr[:NSH] = trgb_np[sl]
        trc = tr.reshape(NT, 128, 3).transpose(1, 0, 2).reshape(128, NT * 3)

        ssl = slice(c * SHW, (c + 1) * SHW)
        lsl = slice(c * LSH, (c + 1) * LSH)
        in_maps.append(dict(
            common,
            t16=np.ascontiguousarray(T),
            trgb=np.ascontiguousarray(trc),
            rgbsh=np.ascontiguousarray(rgbk[:, ssl]),
            keepsh=np.ascontiguousarray(keepk[:, ssl]),
            predsh=np.ascontiguousarray(pred[lsl].reshape(1, LSH)),
            ktgtsh=np.ascontiguousarray(ktgt[lsl].reshape(1, LSH)),
        ))
    return in_maps


_CACHE = {}


def kernel(pred_F, cand_xyz, cand_rgb, tgt_xyz, tgt_rgb, keep_target,
           points_num=8192, **_ignored):
    in_maps = _host_prep(pred_F, cand_xyz, cand_rgb, tgt_xyz, tgt_rgb,
                         keep_target, points_num)
    if "nc" not in _CACHE:
        _CACHE["nc"] = _build_nc()
    res = run_bass_kernel_spmd(_CACHE["nc"], in_maps,
                               core_ids=list(range(NCORES)))
    tot = np.zeros(2, np.float32)
    for c in range(NCORES):
        tot += np.asarray(res.results[c]["out"], np.float32).reshape(2)
    return tot


if __name__ == "__main__":
    import reference as R
    inputs = R.setup_inputs()
    inputs = {kk: np.asarray(vv) if not np.isscalar(vv) else vv
              for kk, vv in inputs.items()}
    out = kernel(**inputs)
    print("kernel out:", out)
